# revision 1
# baseline (speedup 1.0000x reference)
"""RNN-T joint network kernel for Trainium2 (8 NeuronCores, SPMD).

out[b,t,u,v] = (enc[b,t] @ W_enc.T)[v] + (dec[b,u] @ W_dec.T)[v]

Shapes: enc (4,512,512), dec (4,128,512), W (1024,1024) -> out (4,512,128,1024) f32 (1 GiB).

Strategy: shard T across the 8 cores (64 rows each). The 1 GiB output write
is the roofline (~375us/core at ~358 GB/s HBM-per-NC), so the kernel keeps
compute far under that:
  - host pre-transposes all inputs to contraction-major, so the small
    projection matmuls need no on-device transposes (fp32, exact).
  - the (T,U,V) broadcast-add is done in a v-on-partitions layout where the
    encoder term is a per-partition scalar -> DVE tensor_scalar runs at
    2 elem/cycle/lane fp32 (vs 1x for tensor_tensor), with ~30% of tiles
    offloaded to the scalar engine (Identity activation with AP bias).
  - output is written in device layout (B, V, T_loc, U) so every DMA line is
    8 KB contiguous; the host transposes back when gathering.
"""

import sys

if "/opt/trn_rl_repo" not in sys.path:
    sys.path.insert(0, "/opt/trn_rl_repo")

import numpy as np

# Problem shape (hardcoded per contract)
B, T, U, D, V = 4, 512, 128, 512, 1024
N_CORES = 8
P = 128

T_LOC = T // N_CORES          # 64 t-rows per core
TOK = B * T_LOC               # 256 (b,t) rows per core
KT = D // P                   # 4 contraction tiles
VT = V // P                   # 8 v tiles
T_CHUNK = 32                  # t rows per staging tile / output DMA
N_TCH = T_LOC // T_CHUNK      # 4 chunks
BU = B * U                    # 512

_CACHE: dict = {}


def _emit(tc, aps, mybir, act_frac_num=3, act_frac_den=10):
    """Emit the per-core Tile program.

    aps: dict with encT (D,TOK), decT (D,BU), wencT (D,V), wdecT (D,V),
    out (B, VT, P, N_TCH, T_CHUNK*U).
    """
    from contextlib import ExitStack

    nc = tc.nc
    f32 = mybir.dt.float32
    encT, decT, wencT, wdecT, out = (
        aps["encT"], aps["decT"], aps["wencT"], aps["wdecT"], aps["out"],
    )
    b_, vt, p_, ntch, chunk = out.shape
    tok_loc = encT.shape[1] // b_      # t rows per core
    bu = decT.shape[1]
    u_ = bu // b_
    kt = encT.shape[0] // P
    t_chunk = chunk // u_

    with ExitStack() as ctx:
        const = ctx.enter_context(tc.tile_pool(name="const", bufs=1))
        psum = ctx.enter_context(tc.tile_pool(name="psum", bufs=4, space="PSUM"))
        stage = ctx.enter_context(tc.tile_pool(name="stage", bufs=4))

        # --- input loads, critical-path first ---
        # Each logical tensor is loaded with ONE large DMA (k-tiles packed
        # side-by-side in the SBUF free dim) -- large transfers keep the
        # descriptor overhead near zero. Order: the ~2 MB "minimal set"
        # (m=0 weight columns + dec + enc) first, so small early (b=0-only)
        # m=0 projections can start the output stream at ~17us while the
        # remaining 3.5 MB of weight columns stream in underneath.
        def load(src, lo, hi, tag):
            """One DMA: src[:, lo:hi] (D x w) -> SBUF [P, kt*w], free=(k, col)."""
            w = hi - lo
            t = const.tile([P, kt * w], f32, tag=tag)
            nc.sync.dma_start(
                out=t[:].rearrange("p (k c) -> p k c", c=w),
                in_=src[:, lo:hi].rearrange("(k p) c -> p k c", p=P),
            )
            return t

        wdec_m0 = load(wdecT, 0, P, "wdec0")     # [P, kt*128]
        dec_t = load(decT, 0, bu, "dec")         # [P, kt*512]
        wenc_m0 = load(wencT, 0, P, "wenc0")     # [P, kt*128]
        enc_t = load(encT, 0, tokw_g := encT.shape[1], "enc")  # [P, kt*tokw]

        def project(mm_groups, width, tag, on_vector):
            """mm_groups: (lhs_tile, lhs_w, lhs_lo, rhs_tile, rhs_w, rhs_lo, rhs_n, out_lo)."""
            ps = psum.tile([P, width], f32, tag="ps" + tag[0])
            for lhs, lhs_w, lhs_lo, rhs, rhs_w, rhs_lo, rhs_n, out_lo in mm_groups:
                for k in range(kt):
                    nc.tensor.matmul(
                        ps[:, out_lo : out_lo + rhs_n],
                        lhsT=lhs[:, k * lhs_w + lhs_lo : k * lhs_w + lhs_lo + P],
                        rhs=rhs[:, k * rhs_w + rhs_lo : k * rhs_w + rhs_lo + rhs_n],
                        start=(k == 0),
                        stop=(k == kt - 1),
                    )
            sb = const.tile([P, width], f32, tag=tag)
            if on_vector:
                nc.vector.tensor_copy(out=sb[:], in_=ps[:])
            else:
                nc.scalar.activation(sb[:], ps[:], mybir.ActivationFunctionType.Copy)
            return sb

        # early (b=0-only) m=0 projections gate the first output chunks
        dproj0a = project([(wdec_m0, P, 0, dec_t, bu, 0, u_, 0)], u_, "dproj0a", True)
        eproj0a = project(
            [(wenc_m0, P, 0, enc_t, tokw_g, 0, tok_loc, 0)], tok_loc, "eproj0a", False
        )

        def emit_chunk(S_dst, dslice, eproj_tile, tok0, opi):
            for tt in range(t_chunk):
                col = eproj_tile[:, tok0 + tt : tok0 + tt + 1]
                dst = S_dst[:, tt * u_ : (tt + 1) * u_]
                if (opi * act_frac_num) % act_frac_den < act_frac_num:
                    nc.scalar.activation(
                        dst, dslice, mybir.ActivationFunctionType.Identity, bias=col
                    )
                else:
                    nc.vector.tensor_scalar_add(out=dst, in0=dslice, scalar1=col)
                opi += 1
            return opi

        opi = 0
        for tch in range(ntch):  # m=0, b=0 from the early projections
            S = stage.tile([P, chunk], f32, tag="stage")
            opi = emit_chunk(S, dproj0a[:, :u_], eproj0a, tch * t_chunk, opi)
            nc.sync.dma_start(out=out[0, 0, :, tch, :], in_=S[:])

        # --- remaining weight columns + full projections ---
        wr_w = wdecT.shape[1] - P
        wdec_r = load(wdecT, P, wdecT.shape[1], "wdecr")   # [P, kt*896]
        wenc_r = load(wencT, P, wencT.shape[1], "wencr")

        dproj, eproj = [], []
        tokw = encT.shape[1]
        for m in range(vt):
            wd = (wdec_m0, P, 0) if m == 0 else (wdec_r, wr_w, (m - 1) * P)
            we = (wenc_m0, P, 0) if m == 0 else (wenc_r, wr_w, (m - 1) * P)
            dproj.append(
                project(
                    [(wd[0], wd[1], wd[2], dec_t, bu, 0, bu, 0)], bu, f"dproj{m}", True
                )
            )
            eproj.append(
                project(
                    [(we[0], we[1], we[2], enc_t, tokw, 0, tokw, 0)],
                    tokw,
                    f"eproj{m}",
                    False,
                )
            )

        # --- broadcast-add main loop (m=0/b=0 already emitted above) ---
        for m in range(vt):
            for b in range(b_):
                if m == 0 and b == 0:
                    continue
                dslice = dproj[m][:, b * u_ : (b + 1) * u_]
                for tch in range(ntch):
                    S = stage.tile([P, chunk], f32, tag="stage")
                    opi = emit_chunk(S, dslice, eproj[m], b * tok_loc + tch * t_chunk, opi)
                    nc.sync.dma_start(out=out[b, m, :, tch, :], in_=S[:])


def build_bass(num_devices=N_CORES):
    """Build + compile the SPMD Bass program (cached)."""
    key = ("nc", num_devices)
    if key in _CACHE:
        return _CACHE[key]
    import concourse.bacc as bacc
    import concourse.tile as tile
    from concourse import mybir

    nc = bacc.Bacc(
        "TRN2",
        target_bir_lowering=False,
        debug=False,
        num_devices=num_devices,
    )
    f32 = mybir.dt.float32
    aps = {
        "encT": nc.dram_tensor("encT", [D, TOK], f32, kind="ExternalInput").ap(),
        "decT": nc.dram_tensor("decT", [D, BU], f32, kind="ExternalInput").ap(),
        "wencT": nc.dram_tensor("wencT", [D, V], f32, kind="ExternalInput").ap(),
        "wdecT": nc.dram_tensor("wdecT", [D, V], f32, kind="ExternalInput").ap(),
        "out": nc.dram_tensor(
            "out", [B, VT, P, N_TCH, T_CHUNK * U], f32, kind="ExternalOutput"
        ).ap(),
    }
    with tile.TileContext(nc) as tc:
        _emit(tc, aps, mybir)
    nc.compile()
    _CACHE[key] = nc
    return nc


def make_in_maps(encoder_outputs, decoder_outputs, fc_weight):
    enc = np.ascontiguousarray(encoder_outputs, dtype=np.float32)
    dec = np.ascontiguousarray(decoder_outputs, dtype=np.float32)
    w = np.ascontiguousarray(fc_weight, dtype=np.float32)
    decT = np.ascontiguousarray(dec.reshape(BU, D).T)
    wencT = np.ascontiguousarray(w[:, :D].T)
    wdecT = np.ascontiguousarray(w[:, D:].T)
    in_maps = []
    for c in range(N_CORES):
        enc_c = enc[:, c * T_LOC : (c + 1) * T_LOC, :].reshape(TOK, D)
        in_maps.append(
            {
                "encT": np.ascontiguousarray(enc_c.T),
                "decT": decT,
                "wencT": wencT,
                "wdecT": wdecT,
            }
        )
    return in_maps


def assemble(results):
    """results: list of per-core {"out": (B,VT,P,N_TCH,T_CHUNK*U)} -> (B,T,U,V)."""
    full = np.empty((B, T, U, V), dtype=np.float32)
    for c in range(N_CORES):
        arr = results[c]["out"].reshape(B, V, T_LOC, U)
        full[:, c * T_LOC : (c + 1) * T_LOC] = arr.transpose(0, 2, 3, 1)
    return full


def kernel(encoder_outputs, decoder_outputs, fc_weight):
    from concourse.bass_utils import run_bass_kernel_spmd

    nc = build_bass()
    in_maps = make_in_maps(encoder_outputs, decoder_outputs, fc_weight)
    res = run_bass_kernel_spmd(nc, in_maps, list(range(N_CORES)))
    return assemble(res.results)



# revision 6
# speedup vs baseline: 1.7888x; 1.7888x over previous
"""RNN-T joint network kernel for Trainium2 (8 NeuronCores, SPMD).

out[b,t,u,v] = (enc[b,t] @ W_enc.T)[v] + (dec[b,u] @ W_dec.T)[v]

Shapes: enc (4,512,512), dec (4,128,512), W (1024,1024) -> out (4,512,128,1024) f32 (1 GiB).

Strategy: shard T across the 8 cores (64 rows each). The 1 GiB output write
is the roofline (~375us/core at ~358 GB/s HBM-per-NC), so the kernel keeps
compute far under that:
  - host pre-transposes all inputs to contraction-major, so the small
    projection matmuls need no on-device transposes (fp32, exact).
  - the (T,U,V) broadcast-add is done in a v-on-partitions layout where the
    encoder term is a per-partition scalar -> DVE tensor_scalar runs at
    2 elem/cycle/lane fp32 (vs 1x for tensor_tensor), with ~30% of tiles
    offloaded to the scalar engine (Identity activation with AP bias).
  - output is written in device layout (B, V, T_loc, U) so every DMA line is
    8 KB contiguous; the host transposes back when gathering.
"""

import sys

if "/opt/trn_rl_repo" not in sys.path:
    sys.path.insert(0, "/opt/trn_rl_repo")

import numpy as np

# Problem shape (hardcoded per contract)
B, T, U, D, V = 4, 512, 128, 512, 1024
N_CORES = 8
P = 128

T_LOC = T // N_CORES          # 64 t-rows per core
TOK = B * T_LOC               # 256 (b,t) rows per core
KT = D // P                   # 4 contraction tiles
VT = V // P                   # 8 v tiles
T_CHUNK = 32                  # t rows per staging tile / output DMA
N_TCH = T_LOC // T_CHUNK      # 4 chunks
BU = B * U                    # 512

_CACHE: dict = {}


def _emit(tc, aps, mybir, act_frac_num=3, act_frac_den=10):
    """Emit the per-core Tile program.

    aps: dict with encT (D,TOK), decT (D,BU), wencT (D,V), wdecT (D,V),
    out (B, VT, P, N_TCH, T_CHUNK*U).
    """
    from contextlib import ExitStack

    nc = tc.nc
    f32 = mybir.dt.float32
    bf16 = mybir.dt.bfloat16
    encT, decT, wencT, wdecT, out = (
        aps["encT"], aps["decT"], aps["wencT"], aps["wdecT"], aps["out"],
    )
    b_, vt, p_, ntch, chunk = out.shape
    tok_loc = encT.shape[1] // b_      # t rows per core
    bu = decT.shape[1]
    u_ = bu // b_
    kt = encT.shape[0] // P
    t_chunk = chunk // u_

    with ExitStack() as ctx:
        const = ctx.enter_context(tc.tile_pool(name="const", bufs=1))
        psum = ctx.enter_context(tc.tile_pool(name="psum", bufs=4, space="PSUM"))
        stage = ctx.enter_context(tc.tile_pool(name="stage", bufs=4))

        # --- input loads, critical-path first ---
        # Each logical tensor is loaded with ONE large DMA (k-tiles packed
        # side-by-side in the SBUF free dim) -- large transfers keep the
        # descriptor overhead near zero. Order: the ~2 MB "minimal set"
        # (m=0 weight columns + dec + enc) first, so small early (b=0-only)
        # m=0 projections can start the output stream at ~17us while the
        # remaining 3.5 MB of weight columns stream in underneath.
        def load(src, lo, hi, tag):
            """One DMA: src[:, lo:hi] (D x w) -> SBUF [P, kt*w], free=(k, col)."""
            w = hi - lo
            t = const.tile([P, kt * w], f32, tag=tag)
            nc.sync.dma_start(
                out=t[:].rearrange("p (k c) -> p k c", c=w),
                in_=src[:, lo:hi].rearrange("(k p) c -> p k c", p=P),
            )
            return t

        wdec_m0 = load(wdecT, 0, P, "wdec0")     # [P, kt*128]
        dec_t = load(decT, 0, bu, "dec")         # [P, kt*512]
        wenc_m0 = load(wencT, 0, P, "wenc0")     # [P, kt*128]
        enc_t = load(encT, 0, tokw_g := encT.shape[1], "enc")  # [P, kt*tokw]

        def project(mm_groups, width, tag, on_vector):
            """mm_groups: (lhs_tile, lhs_w, lhs_lo, rhs_tile, rhs_w, rhs_lo, rhs_n, out_lo)."""
            ps = psum.tile([P, width], f32, tag="ps" + tag[0])
            for lhs, lhs_w, lhs_lo, rhs, rhs_w, rhs_lo, rhs_n, out_lo in mm_groups:
                for k in range(kt):
                    nc.tensor.matmul(
                        ps[:, out_lo : out_lo + rhs_n],
                        lhsT=lhs[:, k * lhs_w + lhs_lo : k * lhs_w + lhs_lo + P],
                        rhs=rhs[:, k * rhs_w + rhs_lo : k * rhs_w + rhs_lo + rhs_n],
                        start=(k == 0),
                        stop=(k == kt - 1),
                    )
            sb = const.tile([P, width], f32, tag=tag)
            if on_vector:
                nc.vector.tensor_copy(out=sb[:], in_=ps[:])
            else:
                nc.scalar.activation(sb[:], ps[:], mybir.ActivationFunctionType.Copy)
            return sb

        # early (b=0-only) m=0 projections gate the first output chunks
        dproj0a = project([(wdec_m0, P, 0, dec_t, bu, 0, u_, 0)], u_, "dproj0a", True)
        eproj0a = project(
            [(wenc_m0, P, 0, enc_t, tokw_g, 0, tok_loc, 0)], tok_loc, "eproj0a", False
        )

        def emit_chunk(S_dst, dslice, eproj_tile, tok0, opi):
            for tt in range(t_chunk):
                col = eproj_tile[:, tok0 + tt : tok0 + tt + 1]
                dst = S_dst[:, tt * u_ : (tt + 1) * u_]
                if (opi * act_frac_num) % act_frac_den < act_frac_num:
                    nc.scalar.activation(
                        dst, dslice, mybir.ActivationFunctionType.Identity, bias=col
                    )
                else:
                    nc.vector.tensor_scalar_add(out=dst, in0=dslice, scalar1=col)
                opi += 1
            return opi

        opi = 0
        for tch in range(ntch):  # m=0, b=0 from the early projections
            S = stage.tile([P, chunk], bf16, tag="stage")
            opi = emit_chunk(S, dproj0a[:, :u_], eproj0a, tch * t_chunk, opi)
            nc.sync.dma_start(out=out[0, 0, :, tch, :], in_=S[:])

        # --- remaining weight columns + full projections ---
        wr_w = wdecT.shape[1] - P
        wdec_r = load(wdecT, P, wdecT.shape[1], "wdecr")   # [P, kt*896]
        wenc_r = load(wencT, P, wencT.shape[1], "wencr")

        dproj, eproj = [], []
        tokw = encT.shape[1]
        for m in range(vt):
            wd = (wdec_m0, P, 0) if m == 0 else (wdec_r, wr_w, (m - 1) * P)
            we = (wenc_m0, P, 0) if m == 0 else (wenc_r, wr_w, (m - 1) * P)
            dproj.append(
                project(
                    [(wd[0], wd[1], wd[2], dec_t, bu, 0, bu, 0)], bu, f"dproj{m}", True
                )
            )
            eproj.append(
                project(
                    [(we[0], we[1], we[2], enc_t, tokw, 0, tokw, 0)],
                    tokw,
                    f"eproj{m}",
                    False,
                )
            )

        # --- broadcast-add main loop (m=0/b=0 already emitted above) ---
        for m in range(vt):
            for b in range(b_):
                if m == 0 and b == 0:
                    continue
                dslice = dproj[m][:, b * u_ : (b + 1) * u_]
                for tch in range(ntch):
                    S = stage.tile([P, chunk], bf16, tag="stage")
                    opi = emit_chunk(S, dslice, eproj[m], b * tok_loc + tch * t_chunk, opi)
                    nc.sync.dma_start(out=out[b, m, :, tch, :], in_=S[:])


def build_bass(num_devices=N_CORES):
    """Build + compile the SPMD Bass program (cached)."""
    key = ("nc", num_devices)
    if key in _CACHE:
        return _CACHE[key]
    import concourse.bacc as bacc
    import concourse.tile as tile
    from concourse import mybir

    nc = bacc.Bacc(
        "TRN2",
        target_bir_lowering=False,
        debug=False,
        num_devices=num_devices,
    )
    f32 = mybir.dt.float32
    aps = {
        "encT": nc.dram_tensor("encT", [D, TOK], f32, kind="ExternalInput").ap(),
        "decT": nc.dram_tensor("decT", [D, BU], f32, kind="ExternalInput").ap(),
        "wencT": nc.dram_tensor("wencT", [D, V], f32, kind="ExternalInput").ap(),
        "wdecT": nc.dram_tensor("wdecT", [D, V], f32, kind="ExternalInput").ap(),
        "out": nc.dram_tensor(
            "out", [B, VT, P, N_TCH, T_CHUNK * U], mybir.dt.bfloat16,
            kind="ExternalOutput"
        ).ap(),
    }
    with tile.TileContext(nc) as tc:
        _emit(tc, aps, mybir)
    nc.compile()
    _CACHE[key] = nc
    return nc


def make_in_maps(encoder_outputs, decoder_outputs, fc_weight):
    enc = np.ascontiguousarray(encoder_outputs, dtype=np.float32)
    dec = np.ascontiguousarray(decoder_outputs, dtype=np.float32)
    w = np.ascontiguousarray(fc_weight, dtype=np.float32)
    decT = np.ascontiguousarray(dec.reshape(BU, D).T)
    wencT = np.ascontiguousarray(w[:, :D].T)
    wdecT = np.ascontiguousarray(w[:, D:].T)
    in_maps = []
    for c in range(N_CORES):
        enc_c = enc[:, c * T_LOC : (c + 1) * T_LOC, :].reshape(TOK, D)
        in_maps.append(
            {
                "encT": np.ascontiguousarray(enc_c.T),
                "decT": decT,
                "wencT": wencT,
                "wdecT": wdecT,
            }
        )
    return in_maps


def assemble(results):
    """results: list of per-core {"out": (B,VT,P,N_TCH,T_CHUNK*U)} -> (B,T,U,V)."""
    full = np.empty((B, T, U, V), dtype=np.float32)
    for c in range(N_CORES):
        arr = np.asarray(results[c]["out"]).astype(np.float32).reshape(B, V, T_LOC, U)
        full[:, c * T_LOC : (c + 1) * T_LOC] = arr.transpose(0, 2, 3, 1)
    return full


def kernel(encoder_outputs, decoder_outputs, fc_weight):
    from concourse.bass_utils import run_bass_kernel_spmd

    nc = build_bass()
    in_maps = make_in_maps(encoder_outputs, decoder_outputs, fc_weight)
    res = run_bass_kernel_spmd(nc, in_maps, list(range(N_CORES)))
    return assemble(res.results)



# revision 7
# speedup vs baseline: 1.8190x; 1.0169x over previous
"""RNN-T joint network kernel for Trainium2 (8 NeuronCores, SPMD) — v3.

out[b,t,u,v] = (enc[b,t] @ W_enc.T)[v] + (dec[b,u] @ W_dec.T)[v]

Shapes: enc (4,512,512), dec (4,128,512), W (1024,1024) -> out (4,512,128,1024).

Strategy (measured-on-HW numbers per NeuronCore):
  - output written as bf16 (rel err ~4e-3 << 2e-2 gate): HBM write/core
    drops 134MB -> 67MB; DMA roofline ~195us/core at ~358 GB/s.
  - inputs pre-cast to bf16 on host: input DMA 5.8 -> 2.9MB, full bf16
    PE rate for the projection matmuls (PSUM accumulates f32).
  - the (T,U,V) broadcast-add runs on TWO engines in parallel, sized by
    measured per-slab cost (slab = one (m,b): [128v x 64t x 128u]):
      * DVE:  one tensor_tensor with stride-0 broadcast APs, 1x mode:
              (58+8192)/0.96GHz = 8.7us/slab            -> 22 slabs
      * ScalarE: 64 per-t Identity-activations (bias = eproj col),
              ~300ns effective each -> ~19.3us/slab     -> 10 slabs
    GpSimd is deliberately NOT used: its tensor_tensor slows concurrent
    DVE ops 2.6x (measured 8.7 -> 22.4us) — net negative.
  - dproj is consumed directly from PSUM (no SBUF copy); eproj is copied
    to SBUF on ScalarE because activation bias APs must live in SBUF.
  - slab (0,0) is split into 4 sub-ops so the first output DMA issues
    ~6us earlier; every dma_start is striped across all 16 DMA engines
    by the runtime, so one DMA per slab is optimal.
  - output DMA per slab is [128p, 16KB contiguous] in device layout
    (B, VT, P, T_LOC*U); host transposes back when gathering.
"""

import sys

if "/opt/trn_rl_repo" not in sys.path:
    sys.path.insert(0, "/opt/trn_rl_repo")

import numpy as np

# Problem shape (hardcoded per contract)
B, T, U, D, V = 4, 512, 128, 512, 1024
N_CORES = 8
P = 128

T_LOC = T // N_CORES          # 64 t-rows per core
TOK = B * T_LOC               # 256 (b,t) rows per core
KT = D // P                   # 4 contraction tiles
VT = V // P                   # 8 v tiles
BU = B * U                    # 512
CH = T_LOC * U                # 8192 elems per (m,b) slab

# engine per slab index s = m*4 + b: 23 DVE / 9 ScalarE, interleaved
ENGINE_OF_SLAB = list(
    "DADDDADDDADDADDD"
    "ADDADDDADDADDDAD"
)

_CACHE: dict = {}


def _emit(tc, aps, mybir):
    """Emit the per-core Tile program.

    aps: encT (D,TOK), decT (D,BU), wencT (D,V), wdecT (D,V) all bf16;
    out (B, VT, P, CH) bf16.
    """
    from contextlib import ExitStack

    nc = tc.nc
    f32 = mybir.dt.float32
    bf16 = mybir.dt.bfloat16
    encT, decT, wencT, wdecT, out = (
        aps["encT"], aps["decT"], aps["wencT"], aps["wdecT"], aps["out"],
    )
    b_, vt, p_, ch = out.shape
    tok = encT.shape[1]
    tl = tok // b_                     # 64 t rows per (core, b)
    bu = decT.shape[1]
    u_ = bu // b_                      # 128
    kt = encT.shape[0] // P

    with ExitStack() as ctx:
        const = ctx.enter_context(tc.tile_pool(name="const", bufs=1))
        psum = ctx.enter_context(tc.tile_pool(name="psum", bufs=4, space="PSUM"))
        stage_d = ctx.enter_context(tc.tile_pool(name="stage_d", bufs=4))
        stage_a = ctx.enter_context(tc.tile_pool(name="stage_a", bufs=2))

        # --- input loads, critical-path first ---
        def load(src, lo, hi, tag):
            """One DMA: src[:, lo:hi] (D x w) -> SBUF [P, kt*w], free=(k, col)."""
            w = hi - lo
            t = const.tile([P, kt * w], bf16, tag=tag)
            nc.sync.dma_start(
                out=t[:].rearrange("p (k c) -> p k c", c=w),
                in_=src[:, lo:hi].rearrange("(k p) c -> p k c", p=P),
            )
            return t

        wdec_m0 = load(wdecT, 0, P, "wdec0")     # [P, kt*128]
        dec_t = load(decT, 0, bu, "dec")         # [P, kt*512]
        wenc_m0 = load(wencT, 0, P, "wenc0")     # [P, kt*128]
        enc_t = load(encT, 0, tok, "enc")        # [P, kt*256]

        def matmuls(lhs, lhs_w, lhs_lo, rhs, rhs_w, rhs_n, pstag):
            ps = psum.tile([P, rhs_n], f32, tag=pstag)
            for k in range(kt):
                nc.tensor.matmul(
                    ps[:],
                    lhsT=lhs[:, k * lhs_w + lhs_lo : k * lhs_w + lhs_lo + P],
                    rhs=rhs[:, k * rhs_w : k * rhs_w + rhs_n],
                    start=(k == 0),
                    stop=(k == kt - 1),
                )
            return ps

        def emit_slab(eng, dslice, eslice, m, b, nsplit=2):
            """out[p,(t,u)] = eproj[p,t] + dproj[p,u] on the given engine."""
            if eng == "D":
                S = stage_d.tile([P, ch], bf16, tag="stage_d")
                step = tl // nsplit
                for i in range(nsplit):
                    nc.vector.tensor_tensor(
                        out=S[:, i * step * u_ : (i + 1) * step * u_].rearrange(
                            "p (t u) -> p t u", u=u_
                        ),
                        in0=dslice.unsqueeze(1).to_broadcast([P, step, u_]),
                        in1=eslice[:, i * step : (i + 1) * step]
                        .unsqueeze(2)
                        .to_broadcast([P, step, u_]),
                        op=mybir.AluOpType.add,
                    )
                    nc.sync.dma_start(
                        out=out[b, m, :, i * step * u_ : (i + 1) * step * u_],
                        in_=S[:, i * step * u_ : (i + 1) * step * u_],
                    )
            else:  # ScalarE: one Identity-activation per t, bias = eproj col
                S = stage_a.tile([P, ch], bf16, tag="stage_a")
                half = tl // 2
                for h in range(2):
                    for tt in range(h * half, (h + 1) * half):
                        nc.scalar.activation(
                            S[:, tt * u_ : (tt + 1) * u_],
                            dslice,
                            mybir.ActivationFunctionType.Identity,
                            bias=eslice[:, tt : tt + 1],
                        )
                    nc.sync.dma_start(
                        out=out[b, m, :, h * half * u_ : (h + 1) * half * u_],
                        in_=S[:, h * half * u_ : (h + 1) * half * u_],
                    )

        dproj = [None] * vt
        eproj = [None] * vt

        def do_m(m, first=False):
            wd = (wdec_m0, P, 0) if m == 0 else (wdec_r, wr_w, (m - 1) * P)
            we = (wenc_m0, P, 0) if m == 0 else (wenc_r, wr_w, (m - 1) * P)
            ps_d = matmuls(wd[0], wd[1], wd[2], dec_t, bu, bu, "psd")
            ps_e = matmuls(we[0], we[1], we[2], enc_t, tok, tok, "pse")
            # both PSUM->SBUF copies on DVE: ~0.4-0.7us each, they slot into
            # the DVE queue between slab batches without delaying ScalarE
            # (whose queue stays pure slab work).
            sb_d = const.tile([P, bu], f32, tag=f"dproj{m}")
            nc.vector.tensor_copy(out=sb_d[:], in_=ps_d[:])
            sb_e = const.tile([P, tok], f32, tag=f"eproj{m}")
            nc.vector.tensor_copy(out=sb_e[:], in_=ps_e[:])
            dproj[m], eproj[m] = sb_d, sb_e
            for b in range(b_):
                emit_slab(
                    ENGINE_OF_SLAB[m * b_ + b],
                    dproj[m][:, b * u_ : (b + 1) * u_],
                    eproj[m][:, b * tl : (b + 1) * tl],
                    m, b,
                    nsplit=4 if (first and b == 0) else 2,
                )

        # m=0 runs off the small early weight tiles, before the big loads
        do_m(0, first=True)

        wr_w = wdecT.shape[1] - P
        wdec_r = load(wdecT, P, wdecT.shape[1], "wdecr")   # [P, kt*896]
        wenc_r = load(wencT, P, wencT.shape[1], "wencr")
        for m in range(1, vt):
            do_m(m)


def build_bass(num_devices=N_CORES):
    """Build + compile the SPMD Bass program (cached)."""
    key = ("nc", num_devices)
    if key in _CACHE:
        return _CACHE[key]
    import concourse.bacc as bacc
    import concourse.tile as tile
    from concourse import mybir

    nc = bacc.Bacc(
        "TRN2",
        target_bir_lowering=False,
        debug=False,
        num_devices=num_devices,
    )
    bf16 = mybir.dt.bfloat16
    aps = {
        "encT": nc.dram_tensor("encT", [D, TOK], bf16, kind="ExternalInput").ap(),
        "decT": nc.dram_tensor("decT", [D, BU], bf16, kind="ExternalInput").ap(),
        "wencT": nc.dram_tensor("wencT", [D, V], bf16, kind="ExternalInput").ap(),
        "wdecT": nc.dram_tensor("wdecT", [D, V], bf16, kind="ExternalInput").ap(),
        "out": nc.dram_tensor(
            "out", [B, VT, P, CH], bf16, kind="ExternalOutput"
        ).ap(),
    }
    with tile.TileContext(nc) as tc:
        _emit(tc, aps, mybir)
    nc.compile()
    _CACHE[key] = nc
    return nc


def make_in_maps(encoder_outputs, decoder_outputs, fc_weight):
    import ml_dtypes

    bf = ml_dtypes.bfloat16
    enc = np.asarray(encoder_outputs, dtype=np.float32)
    dec = np.asarray(decoder_outputs, dtype=np.float32)
    w = np.asarray(fc_weight, dtype=np.float32)
    decT = np.ascontiguousarray(dec.reshape(BU, D).T.astype(bf))
    wencT = np.ascontiguousarray(w[:, :D].T.astype(bf))
    wdecT = np.ascontiguousarray(w[:, D:].T.astype(bf))
    in_maps = []
    for c in range(N_CORES):
        enc_c = enc[:, c * T_LOC : (c + 1) * T_LOC, :].reshape(TOK, D)
        in_maps.append(
            {
                "encT": np.ascontiguousarray(enc_c.T.astype(bf)),
                "decT": decT,
                "wencT": wencT,
                "wdecT": wdecT,
            }
        )
    return in_maps


def assemble(results):
    """results: list of per-core {"out": (B,VT,P,CH)} -> (B,T,U,V) f32."""
    full = np.empty((B, T, U, V), dtype=np.float32)
    for c in range(N_CORES):
        arr = np.asarray(results[c]["out"]).astype(np.float32)
        arr = arr.reshape(B, V, T_LOC, U)
        full[:, c * T_LOC : (c + 1) * T_LOC] = arr.transpose(0, 2, 3, 1)
    return full


def kernel(encoder_outputs, decoder_outputs, fc_weight):
    from concourse.bass_utils import run_bass_kernel_spmd

    nc = build_bass()
    in_maps = make_in_maps(encoder_outputs, decoder_outputs, fc_weight)
    res = run_bass_kernel_spmd(nc, in_maps, list(range(N_CORES)))
    return assemble(res.results)


# revision 8
# speedup vs baseline: 1.8240x; 1.0027x over previous
"""RNN-T joint network kernel for Trainium2 (8 NeuronCores, SPMD).

out[b,t,u,v] = (enc[b,t] @ W_enc.T)[v] + (dec[b,u] @ W_dec.T)[v]

Shapes: enc (4,512,512), dec (4,128,512), W (1024,1024) -> out (4,512,128,1024).

Strategy (measured-on-HW numbers per NeuronCore):
  - output written as bf16 (rel err ~4e-3 << 2e-2 gate): HBM write/core
    drops 134MB -> 67MB; DMA roofline ~195us/core at ~358 GB/s.
  - inputs pre-cast to bf16 on host: input DMA 5.8 -> 2.9MB, full bf16
    PE rate for the projection matmuls (PSUM accumulates f32).
  - the (T,U,V) broadcast-add runs on TWO engines in parallel, sized by
    measured per-slab cost (slab = one (m,b): [128v x 64t x 128u]):
      * DVE:  tensor_tensor with stride-0 broadcast APs, 1x mode:
              2x(58+4096)/0.96GHz = 8.85us/slab         -> 22 slabs
      * ScalarE: 64 per-t Identity-activations (bias = eproj col),
              ~300ns effective each -> ~19us/slab       -> 10 slabs
    GpSimd is deliberately NOT used: its tensor_tensor slows concurrent
    DVE ops 2.6x (measured 8.7 -> 22.4us) — net negative.
  - projections are copied PSUM->SBUF on DVE (~0.5us each); putting any
    copy on ScalarE stalls the other engine a full Act slab (in-order
    engine queues), which is what regressed v3/v4.
  - per-engine stage pools (a shared ring couples DVE to the slower
    ScalarE slab cadence); slab DMAs are split so the first output DMA
    issues ~14us in and the last one drains a quarter-slab.
  - output DMA per slab is [128p, 16KB contiguous] in device layout
    (B, VT, P, T_LOC*U); host transposes back when gathering.
"""

import sys

if "/opt/trn_rl_repo" not in sys.path:
    sys.path.insert(0, "/opt/trn_rl_repo")

import numpy as np

# Problem shape (hardcoded per contract)
B, T, U, D, V = 4, 512, 128, 512, 1024
N_CORES = 8
P = 128

T_LOC = T // N_CORES          # 64 t-rows per core
TOK = B * T_LOC               # 256 (b,t) rows per core
KT = D // P                   # 4 contraction tiles
VT = V // P                   # 8 v tiles
BU = B * U                    # 512
CH = T_LOC * U                # 8192 elems per (m,b) slab

# engine per slab index s = m*4 + b: 22 DVE / 10 ScalarE, interleaved
ENGINE_OF_SLAB = list(
    "DADDADDDADDADDAD"
    "DADDADDADDADDADD"
)

_CACHE: dict = {}


def _emit(tc, aps, mybir):
    """Emit the per-core Tile program.

    aps: encT (D,TOK), decT (D,BU), wencT (D,V), wdecT (D,V) all bf16;
    out (B, VT, P, CH) bf16.
    """
    from contextlib import ExitStack

    nc = tc.nc
    f32 = mybir.dt.float32
    bf16 = mybir.dt.bfloat16
    encT, decT, wencT, wdecT, out = (
        aps["encT"], aps["decT"], aps["wencT"], aps["wdecT"], aps["out"],
    )
    b_, vt, p_, ch = out.shape
    tok = encT.shape[1]
    tl = tok // b_                     # 64 t rows per (core, b)
    bu = decT.shape[1]
    u_ = bu // b_                      # 128
    kt = encT.shape[0] // P

    with ExitStack() as ctx:
        const = ctx.enter_context(tc.tile_pool(name="const", bufs=1))
        psum = ctx.enter_context(tc.tile_pool(name="psum", bufs=4, space="PSUM"))
        stage_d = ctx.enter_context(tc.tile_pool(name="stage_d", bufs=4))
        stage_a = ctx.enter_context(tc.tile_pool(name="stage_a", bufs=2))

        # --- input loads, critical-path first ---
        def load(src, lo, hi, tag):
            """One DMA: src[:, lo:hi] (D x w) -> SBUF [P, kt*w], free=(k, col)."""
            w = hi - lo
            t = const.tile([P, kt * w], bf16, tag=tag)
            nc.sync.dma_start(
                out=t[:].rearrange("p (k c) -> p k c", c=w),
                in_=src[:, lo:hi].rearrange("(k p) c -> p k c", p=P),
            )
            return t

        wdec_m0 = load(wdecT, 0, P, "wdec0")     # [P, kt*128]
        dec_t = load(decT, 0, bu, "dec")         # [P, kt*512]
        wenc_m0 = load(wencT, 0, P, "wenc0")     # [P, kt*128]
        enc_t = load(encT, 0, tok, "enc")        # [P, kt*256]

        def matmuls(lhs, lhs_w, lhs_lo, rhs, rhs_w, rhs_n, pstag):
            ps = psum.tile([P, rhs_n], f32, tag=pstag)
            for k in range(kt):
                nc.tensor.matmul(
                    ps[:],
                    lhsT=lhs[:, k * lhs_w + lhs_lo : k * lhs_w + lhs_lo + P],
                    rhs=rhs[:, k * rhs_w : k * rhs_w + rhs_n],
                    start=(k == 0),
                    stop=(k == kt - 1),
                )
            return ps

        def emit_slab(eng, dslice, eslice, m, b, nsplit=2):
            """out[p,(t,u)] = eproj[p,t] + dproj[p,u] on the given engine."""
            if eng == "D":
                S = stage_d.tile([P, ch], bf16, tag="stage_d")
                step = tl // nsplit
                for i in range(nsplit):
                    nc.vector.tensor_tensor(
                        out=S[:, i * step * u_ : (i + 1) * step * u_].rearrange(
                            "p (t u) -> p t u", u=u_
                        ),
                        in0=dslice.unsqueeze(1).to_broadcast([P, step, u_]),
                        in1=eslice[:, i * step : (i + 1) * step]
                        .unsqueeze(2)
                        .to_broadcast([P, step, u_]),
                        op=mybir.AluOpType.add,
                    )
                    nc.sync.dma_start(
                        out=out[b, m, :, i * step * u_ : (i + 1) * step * u_],
                        in_=S[:, i * step * u_ : (i + 1) * step * u_],
                    )
            else:  # ScalarE: one Identity-activation per t, bias = eproj col
                S = stage_a.tile([P, ch], bf16, tag="stage_a")
                half = tl // 2
                for h in range(2):
                    for tt in range(h * half, (h + 1) * half):
                        nc.scalar.activation(
                            S[:, tt * u_ : (tt + 1) * u_],
                            dslice,
                            mybir.ActivationFunctionType.Identity,
                            bias=eslice[:, tt : tt + 1],
                        )
                    nc.sync.dma_start(
                        out=out[b, m, :, h * half * u_ : (h + 1) * half * u_],
                        in_=S[:, h * half * u_ : (h + 1) * half * u_],
                    )

        dproj = [None] * vt
        eproj = [None] * vt

        def do_m(m, first=False):
            wd = (wdec_m0, P, 0) if m == 0 else (wdec_r, wr_w, (m - 1) * P)
            we = (wenc_m0, P, 0) if m == 0 else (wenc_r, wr_w, (m - 1) * P)
            ps_d = matmuls(wd[0], wd[1], wd[2], dec_t, bu, bu, "psd")
            ps_e = matmuls(we[0], we[1], we[2], enc_t, tok, tok, "pse")
            # both PSUM->SBUF copies on DVE: ~0.4-0.7us each, they slot into
            # the DVE queue between slab batches without delaying ScalarE
            # (whose queue stays pure slab work).
            sb_d = const.tile([P, bu], f32, tag=f"dproj{m}")
            nc.vector.tensor_copy(out=sb_d[:], in_=ps_d[:])
            sb_e = const.tile([P, tok], f32, tag=f"eproj{m}")
            nc.vector.tensor_copy(out=sb_e[:], in_=ps_e[:])
            dproj[m], eproj[m] = sb_d, sb_e
            for b in range(b_):
                emit_slab(
                    ENGINE_OF_SLAB[m * b_ + b],
                    dproj[m][:, b * u_ : (b + 1) * u_],
                    eproj[m][:, b * tl : (b + 1) * tl],
                    m, b,
                    nsplit=4 if ((first and b == 0) or (m == vt - 1 and b == b_ - 1)) else 2,
                )

        # m=0 runs off the small early weight tiles, before the big loads
        do_m(0, first=True)

        wr_w = wdecT.shape[1] - P
        wdec_r = load(wdecT, P, wdecT.shape[1], "wdecr")   # [P, kt*896]
        wenc_r = load(wencT, P, wencT.shape[1], "wencr")
        for m in range(1, vt):
            do_m(m)


def build_bass(num_devices=N_CORES):
    """Build + compile the SPMD Bass program (cached)."""
    key = ("nc", num_devices)
    if key in _CACHE:
        return _CACHE[key]
    import concourse.bacc as bacc
    import concourse.tile as tile
    from concourse import mybir

    nc = bacc.Bacc(
        "TRN2",
        target_bir_lowering=False,
        debug=False,
        num_devices=num_devices,
    )
    bf16 = mybir.dt.bfloat16
    aps = {
        "encT": nc.dram_tensor("encT", [D, TOK], bf16, kind="ExternalInput").ap(),
        "decT": nc.dram_tensor("decT", [D, BU], bf16, kind="ExternalInput").ap(),
        "wencT": nc.dram_tensor("wencT", [D, V], bf16, kind="ExternalInput").ap(),
        "wdecT": nc.dram_tensor("wdecT", [D, V], bf16, kind="ExternalInput").ap(),
        "out": nc.dram_tensor(
            "out", [B, VT, P, CH], bf16, kind="ExternalOutput"
        ).ap(),
    }
    with tile.TileContext(nc) as tc:
        _emit(tc, aps, mybir)
    nc.compile()
    _CACHE[key] = nc
    return nc


def make_in_maps(encoder_outputs, decoder_outputs, fc_weight):
    import ml_dtypes

    bf = ml_dtypes.bfloat16
    enc = np.asarray(encoder_outputs, dtype=np.float32)
    dec = np.asarray(decoder_outputs, dtype=np.float32)
    w = np.asarray(fc_weight, dtype=np.float32)
    decT = np.ascontiguousarray(dec.reshape(BU, D).T.astype(bf))
    wencT = np.ascontiguousarray(w[:, :D].T.astype(bf))
    wdecT = np.ascontiguousarray(w[:, D:].T.astype(bf))
    in_maps = []
    for c in range(N_CORES):
        enc_c = enc[:, c * T_LOC : (c + 1) * T_LOC, :].reshape(TOK, D)
        in_maps.append(
            {
                "encT": np.ascontiguousarray(enc_c.T.astype(bf)),
                "decT": decT,
                "wencT": wencT,
                "wdecT": wdecT,
            }
        )
    return in_maps


def assemble(results):
    """results: list of per-core {"out": (B,VT,P,CH)} -> (B,T,U,V) f32."""
    full = np.empty((B, T, U, V), dtype=np.float32)
    for c in range(N_CORES):
        arr = np.asarray(results[c]["out"]).astype(np.float32)
        arr = arr.reshape(B, V, T_LOC, U)
        full[:, c * T_LOC : (c + 1) * T_LOC] = arr.transpose(0, 2, 3, 1)
    return full


def kernel(encoder_outputs, decoder_outputs, fc_weight):
    from concourse.bass_utils import run_bass_kernel_spmd

    nc = build_bass()
    in_maps = make_in_maps(encoder_outputs, decoder_outputs, fc_weight)
    res = run_bass_kernel_spmd(nc, in_maps, list(range(N_CORES)))
    return assemble(res.results)
